# revision 42
# baseline (speedup 1.0000x reference)
"""Trainium2 Bass kernel for nn_AttentionBlock (B=4, C=1024, T=2048, H=16, GROUPS=32).

Sharding: 8 cores = 4 batches x 2 head-halves (tensor parallel). Each core
computes, for its batch b and its 8 heads:
  - GroupNorm(x[b]) (full T; stats duplicated between the two cores of a pair)
  - q/k/v for its 8 heads over the full T (no duplicated projection work)
  - masked softmax attention for its 8 heads, full T x T
  - partial proj (contraction over its 512 a-channels) + xn/2 + b_proj/2
  - host sums the two partial outputs of each pair (the "all-reduce")

Numerics: GroupNorm stats in fp32; weights/xn/a in fp8e4m3 (DoubleRow matmuls
for qkv/v/proj, x8 weight prescale); q/k/probabilities bf16. Softmax
denominators come free from the PV matmul (v^T tiles carry a 64-wide block of
1/16 so d/16 lands replicated on psum partitions 64:128); -1/d via a bf16
bit-hack seed + one Newton step on DVE (sign folded into W_proj). q/k
projections for the next head pair are emitted inside the previous attention
block's tail; softmax normalization drips one DVE op per tsb through the next
block. Output is bf16 partials summed on the host in fp32.
"""

import numpy as np
import ml_dtypes

import concourse.bass as bass
import concourse.bacc as bacc_mod
import concourse.tile as tile
import concourse.mybir as mybir
from concourse.bass_utils import run_bass_kernel_spmd

F32 = mybir.dt.float32
F32R = mybir.dt.float32r
BF16 = mybir.dt.bfloat16
FP8 = mybir.dt.float8e4
AF = mybir.ActivationFunctionType
OP = mybir.AluOpType
DR = mybir.MatmulPerfMode.DoubleRow
WSCALE = 8.0          # host pre-scales weights into fp8 normal range
ONES_V = 1.0 / 16.0   # ones block value; makes a_all land in fp8 normal range

B, C, T, H = 4, 1024, 2048, 16
GROUPS = 32
EPS = 1e-5
CH = C // H              # 64
SCALE = float(CH) ** -0.25
NH = H // 2              # 8 heads per core
NPAIR = NH // 2          # 4 local head pairs
NCO = C // 128           # 8 channel blocks
NCA = NH * CH // 128     # 4 local a-channel blocks
NTSB = T // 128          # 16 key/value blocks
NTQB = T // 512          # 4 query sub-blocks per pair
INV_N = 1.0 / (32 * T)   # group size = 32 channels x T

# smalls tile column layout (all [128, 8] blocks)
S_S1, S_S2 = 0, 8
S_MU, S_E2, S_VAR, S_SQ, S_RS, S_A, S_B = 16, 24, 32, 40, 48, 56, 64
S_GNW, S_GNB, S_BQ, S_BK, S_BP = 72, 80, 88, 96, 104
S_A2, S_B2 = 112, 120
S_EPS = 128
S_COLS = 129


def build_nc():
    nc = bacc_mod.Bacc(None, target_bir_lowering=False)
    f = {}
    f["x_t"] = nc.dram_tensor("x_t", [128, NCO, T], BF16, kind="ExternalInput")
    f["mask_t"] = nc.dram_tensor("mask_t", [128, NTSB, T], BF16, kind="ExternalInput")
    f["wq_t"] = nc.dram_tensor("wq_t", [NPAIR, 128, NCO, 128], FP8, kind="ExternalInput")
    f["wk_t"] = nc.dram_tensor("wk_t", [NPAIR, 128, NCO, 128], FP8, kind="ExternalInput")
    f["wv_t"] = nc.dram_tensor("wv_t", [128, NCO, 512], FP8, kind="ExternalInput")
    f["wp_t"] = nc.dram_tensor("wp_t", [NCO, 128, NCA, 128], FP8, kind="ExternalInput")
    f["gnw_t"] = nc.dram_tensor("gnw_t", [128, NCO], F32, kind="ExternalInput")
    f["gnb_t"] = nc.dram_tensor("gnb_t", [128, NCO], F32, kind="ExternalInput")
    f["bqs_t"] = nc.dram_tensor("bqs_t", [128, NPAIR], F32, kind="ExternalInput")
    f["bks_t"] = nc.dram_tensor("bks_t", [128, NPAIR], F32, kind="ExternalInput")
    f["bvb_t"] = nc.dram_tensor("bvb_t", [128, 512], BF16, kind="ExternalInput")
    f["bp_t"] = nc.dram_tensor("bp_t", [128, NCO], F32, kind="ExternalInput")
    f["ind2_t"] = nc.dram_tensor("ind2_t", [128, 128], F32R, kind="ExternalInput")
    out_t = nc.dram_tensor("out_t", [128, NCO, T], BF16, kind="ExternalOutput")

    with tile.TileContext(nc) as tc:
        build_body(nc, tc, f, out_t)
    nc.compile()
    return nc


def build_body(nc, tc, f, out_t):
    import contextlib
    ctx = contextlib.ExitStack()
    with ctx:
        singles = ctx.enter_context(tc.tile_pool(name="singles", bufs=1))
        bigp = ctx.enter_context(tc.tile_pool(name="bigp", bufs=1))
        wqk = ctx.enter_context(tc.tile_pool(name="wqk", bufs=3))
        qpool = ctx.enter_context(tc.tile_pool(name="qpool", bufs=2))
        kpool = ctx.enter_context(tc.tile_pool(name="kpool", bufs=2))
        ppool = ctx.enter_context(tc.tile_pool(name="ppool", bufs=3))
        rpool = ctx.enter_context(tc.tile_pool(name="rpool", bufs=2))
        opool = ctx.enter_context(tc.tile_pool(name="opool", bufs=2))
        psum = ctx.enter_context(tc.tile_pool(name="psum", bufs=4, space="PSUM"))

        # ---- persistent tiles ----
        xnb = singles.tile([128, NCO, T], BF16)      # GroupNorm/2 (residual half)
        xn8 = singles.tile([128, NCO, T], FP8)       # GroupNorm output (fp8, matmul input)
        vt = singles.tile([128, NTSB, NH, 128], BF16)
        a_all = singles.tile([128, NPAIR, T], FP8)
        sm = singles.tile([128, S_COLS], F32)
        gst2 = singles.tile([128, 16], F32R)
        bvb = singles.tile([128, 512], BF16)
        ind2 = singles.tile([128, 128], F32R)
        # wv dies once v-proj ends, well before the second q_pair tile is
        # allocated, so it can ride in the qpool rotation
        wv_all = qpool.tile([128, NCO, 512], FP8, tag="qpair", name="wv_all")

        # raw x shares its SBUF slot with the full-T mask, which only starts
        # loading after x is dead
        xs = bigp.tile([128, NCO, T], BF16, tag="big", name="xs")

        # x alternates across the sync + gpsimd DMA rings (Pool is idle this
        # early) so the scalar ring only carries the small coefficient
        # transfers and Scalar can start the stats squares immediately
        nc.scalar.dma_start(out=ind2, in_=f["ind2_t"][:])
        for co in range(NCO):
            eng = nc.sync if co % 2 == 0 else nc.gpsimd
            eng.dma_start(out=xs[:, co, :], in_=f["x_t"][:, co, :])
        # wv rides the scalar ring behind the tiny coefficient transfers so
        # it never delays the x slices that gate the stats
        nc.scalar.dma_start(out=wv_all, in_=f["wv_t"][:])
        nc.scalar.dma_start(out=sm[:, S_GNW:S_GNW + 8], in_=f["gnw_t"][:])
        nc.scalar.dma_start(out=sm[:, S_GNB:S_GNB + 8], in_=f["gnb_t"][:])
        nc.scalar.dma_start(out=bvb, in_=f["bvb_t"][:])
        nc.scalar.dma_start(out=sm[:, S_BQ:S_BQ + NPAIR], in_=f["bqs_t"][:])
        nc.scalar.dma_start(out=sm[:, S_BK:S_BK + NPAIR], in_=f["bks_t"][:])
        nc.scalar.dma_start(out=sm[:, S_BP:S_BP + 8], in_=f["bp_t"][:])

        nc.vector.memset(sm[:, S_EPS:S_EPS + 1], EPS)

        # ---- GroupNorm stats, processed in two co-halves so the first
        # half's xn8 (and thus the v projection) starts ~10us earlier ----
        # per-channel sums on DVE; per-channel sums of squares on Scalar
        # (Square + accumulator) so the two run concurrently
        trash = kpool.tile([128, T], BF16, tag="kpair", name="trash")
        gps_full = psum.tile([128, 512], F32, tag="acc", bufs=2, name="gps")

        def stats_half(hf):
            o = hf * 4
            sl = slice(o, o + 4)
            for co in range(o, o + 4):
                nc.vector.tensor_reduce(
                    out=sm[:, co:co + 1], in_=xs[:, co, :],
                    axis=mybir.AxisListType.X, op=OP.add,
                )
                nc.scalar.activation(
                    out=trash, in_=xs[:, co, :], func=AF.Square,
                    accum_out=sm[:, 8 + co:9 + co],
                )
            # pack this half's 8 stat columns and aggregate the 32-channel
            # groups via the block-diagonal indicator matmul
            g8 = gst2[:, hf * 8:hf * 8 + 8]
            nc.vector.tensor_copy(out=g8[:, 0:4], in_=sm[:, sl])
            nc.vector.tensor_copy(out=g8[:, 4:8], in_=sm[:, 8 + o:8 + o + 4])
            gps = gps_full[:, hf * 16:hf * 16 + 8]
            nc.tensor.matmul(gps, lhsT=ind2, rhs=g8, start=True, stop=True)

            def c(base):
                return sm[:, base + o:base + o + 4]

            nc.vector.tensor_scalar_mul(c(S_MU), gps[:, 0:4], INV_N)
            nc.vector.tensor_scalar_mul(c(S_E2), gps[:, 4:8], INV_N)
            nc.vector.tensor_tensor(c(S_VAR), c(S_MU), c(S_MU), OP.mult)
            nc.vector.tensor_tensor(c(S_VAR), c(S_E2), c(S_VAR), OP.subtract)
            nc.scalar.activation(out=c(S_SQ), in_=c(S_VAR),
                                 func=AF.Sqrt, bias=sm[:, S_EPS:S_EPS + 1])
            nc.vector.reciprocal(out=c(S_RS), in_=c(S_SQ))
            # A = rstd * gn_w ; Bc = gn_b - mu * A ; halved copies for the
            # residual (each core of a pair contributes xn/2)
            nc.vector.tensor_tensor(c(S_A), c(S_RS), c(S_GNW), OP.mult)
            nc.vector.tensor_tensor(c(S_B), c(S_MU), c(S_A), OP.mult)
            nc.vector.tensor_tensor(c(S_B), c(S_GNB), c(S_B), OP.subtract)
            nc.vector.tensor_scalar_mul(c(S_A2), c(S_A), 0.5)
            nc.vector.tensor_scalar_mul(c(S_B2), c(S_B), 0.5)
            for co in range(o, o + 4):
                if co % 2 == 0:
                    nc.vector.tensor_scalar(
                        out=xn8[:, co, :], in0=xs[:, co, :],
                        scalar1=sm[:, S_A + co:S_A + co + 1],
                        scalar2=sm[:, S_B + co:S_B + co + 1],
                        op0=OP.mult, op1=OP.add,
                    )
                else:
                    nc.scalar.activation(
                        out=xn8[:, co, :], in_=xs[:, co, :], func=AF.Identity,
                        bias=sm[:, S_B + co:S_B + co + 1],
                        scale=sm[:, S_A + co:S_A + co + 1],
                    )

        stats_half(0)
        stats_half(1)
        for co in range(NCO):
            nc.gpsimd.tensor_scalar(
                out=xnb[:, co, :], in0=xs[:, co, :],
                scalar1=sm[:, S_A2 + co:S_A2 + co + 1],
                scalar2=sm[:, S_B2 + co:S_B2 + co + 1],
                op0=OP.mult, op1=OP.add,
            )

        # full-T mask loads into the big slot as soon as x dies; the first
        # chunks ride the scalar ring (free once the smalls are done)
        mask_s = bigp.tile([128, NTSB, T], BF16, tag="big", name="mask_s")
        for mc in range(8):
            eng = nc.scalar if mc < 4 else nc.sync
            eng.dma_start(out=mask_s[:, 2 * mc:2 * mc + 2, :],
                          in_=f["mask_t"][:, 2 * mc:2 * mc + 2, :])

        # ---- v^T for the 8 local heads ----
        # vt[:, tsb, h, 0:64] = v^T block; vt[:, tsb, h, 64:128] = ones so the
        # PV matmul also produces the softmax denominator on partitions 64:128
        for mg in range(4):
            nc.gpsimd.memset(vt[:, 4 * mg:4 * mg + 4, :, CH:128], ONES_V)

        vgroups = []

        def v_group(tbg):
            vps = [psum.tile([128, 512], F32, tag=("acc" if i < 2 else "st"),
                             bufs=(2 if i < 2 else 3), name=f"vps{i}")
                   for i in range(4)]
            for kb2 in range(NCO // 2):
                wv_sl = wv_all[:, 2 * kb2:2 * kb2 + 2, :]
                for i in range(4):
                    tb = tbg * 4 + i
                    nc.tensor.matmul(
                        vps[i],
                        lhsT=xn8[:, 2 * kb2:2 * kb2 + 2, tb * 128:(tb + 1) * 128],
                        rhs=wv_sl,
                        start=(kb2 == 0), stop=(kb2 == NCO // 2 - 1),
                        perf_mode=DR,
                    )
            for i in range(4):
                tb = tbg * 4 + i
                nc.vector.tensor_tensor(
                    out=vt[:, tb, :, 0:CH],
                    in0=vps[i].rearrange("p (h c) -> p h c", c=CH),
                    in1=bvb.rearrange("p (h c) -> p h c", c=CH),
                    op=OP.add,
                )

        # ---- per head-pair: q/k projections then attention ----
        norm_q = []

        def norm_steps(item):
            # -1/d via bf16 bit-hack seed + one Newton step, in cheap DVE
            # ALU ops (the iterative InstReciprocal is 4x slower); the sign
            # is folded into W_proj on the host. One double-width chain
            # covers both heads of the block (accs2 cols 0:512 = head even,
            # 512:1024 = head odd), dripped one op per tsb.
            accs_, hp_, tq_sl_ = item
            I16 = mybir.dt.int16
            h = {}

            # scratch tiles are [128, 1024] sliced at [64:128] so SB+SB
            # operands share the same base partition as accs_[64:128]
            def s_t1():
                # NOT(x - 0x7EF3) == (x - 0x7EF2) * -1 in two's complement,
                # so the seed needs only one arith tensor_scalar
                t1f = rpool.tile([128, 1024], I16, tag="rd", bufs=3, name="t1f")
                h["t1"] = t1f[64:128, :]
                nc.vector.tensor_scalar(
                    out=h["t1"], in0=accs_[64:128, :].bitcast(I16),
                    scalar1=0x7EF2, scalar2=-1,
                    op0=OP.subtract, op1=OP.mult,
                )

            def s_u():
                uf = rpool.tile([128, 1024], BF16, tag="rd", bufs=3, name="uf")
                h["u"] = uf[64:128, :]
                nc.vector.tensor_tensor(out=h["u"], in0=accs_[64:128, :],
                                        in1=h["t1"].bitcast(BF16), op=OP.mult)

            def s_rneg():
                # stt computes (scalar op0 in0) op1 in1 = (2 - u) * r0
                rnegf = rpool.tile([128, 1024], BF16, tag="rd", bufs=3,
                                   name="rnegf")
                h["rneg"] = rnegf[0:64, :]
                nc.vector.scalar_tensor_tensor(
                    out=h["rneg"], in0=h["u"], scalar=2.0,
                    in1=h["t1"].bitcast(BF16),
                    op0=OP.subtract, op1=OP.mult,
                )

            def s_fin0():
                nc.gpsimd.tensor_tensor(
                    out=a_all[0:64, hp_, tq_sl_], in0=accs_[0:64, 0:512],
                    in1=h["rneg"][:, 0:512], op=OP.mult,
                )

            def s_fin1():
                nc.gpsimd.tensor_tensor(
                    out=a_all[64:128, hp_, tq_sl_], in0=accs_[0:64, 512:1024],
                    in1=h["rneg"][:, 512:1024], op=OP.mult,
                )

            return [s_t1, s_u, s_rneg, s_fin0, s_fin1]

        def make_qk(hp):
            """q/k projection for head-pair hp as 5 chunks, emitted inside the
            previous attention block (hidden under the exp/mask pipeline) on a
            dedicated PSUM bank pair."""
            state = {}

            def c_dma():
                wq_sl = wqk.tile([128, NCO, 128], FP8, tag="wqkr", name="wq_sl")
                nc.sync.dma_start(out=wq_sl, in_=f["wq_t"][hp])
                wk_sl = wqk.tile([128, NCO, 128], FP8, tag="wqkr", name="wk_sl")
                nc.sync.dma_start(out=wk_sl, in_=f["wk_t"][hp])
                state["wq"], state["wk"] = wq_sl, wk_sl

            def c_qk(th, qu):
                if th == 0 and qu == 0:
                    state["q"] = qpool.tile([128, T], BF16, tag="qpair",
                                            name="q_pair")
                if th == 0 and qu == 1:
                    state["k"] = kpool.tile([128, T], BF16, tag="kpair",
                                            name="k_pair")
                dst = state["q"] if qu == 0 else state["k"]
                w_sl = state["wq"] if qu == 0 else state["wk"]
                bias = sm[:, S_BQ + hp:S_BQ + hp + 1] if qu == 0 else \
                    sm[:, S_BK + hp:S_BK + hp + 1]
                qps2 = psum.tile([128, 1024], F32, tag="st", bufs=3, name="qps2")
                for tqb2 in range(2):
                    for kb2 in range(NCO // 2):
                        nc.tensor.matmul(
                            qps2[:, tqb2 * 512:(tqb2 + 1) * 512],
                            lhsT=w_sl[:, 2 * kb2:2 * kb2 + 2, :],
                            rhs=xn8[:, 2 * kb2:2 * kb2 + 2,
                                    th * 1024 + tqb2 * 512:
                                    th * 1024 + (tqb2 + 1) * 512],
                            start=(kb2 == 0), stop=(kb2 == NCO // 2 - 1),
                            perf_mode=DR,
                        )
                nc.vector.tensor_scalar(
                    out=dst[:, th * 1024:(th + 1) * 1024], in0=qps2,
                    scalar1=SCALE / WSCALE, scalar2=bias,
                    op0=OP.mult, op1=OP.add,
                )

            return state, [c_dma, lambda: c_qk(0, 0), lambda: c_qk(1, 0),
                           lambda: c_qk(0, 1), lambda: c_qk(1, 1)]

        # v groups with hp0's q/k pieces interleaved so the q/k DVE copies
        # overlap the later v matmul groups
        qk_state, chunks0 = make_qk(0)
        chunks0[0]()
        v_group(0)
        v_group(1)
        chunks0[1]()
        v_group(2)
        chunks0[2]()
        chunks0[3]()
        v_group(3)
        chunks0[4]()

        wp_tiles = {}

        def wp_dma(mb):
            wp_sl = wqk.tile([128, NCA, 128], FP8, tag="wqkr", name="wp_sl")
            nc.sync.dma_start(out=wp_sl, in_=f["wp_t"][mb])
            wp_tiles[mb] = wp_sl

        for hp in range(NPAIR):
            q_pair, k_pair = qk_state["q"], qk_state["k"]
            next_state = None
            for tqb in range(NTQB):
                final_block = (tqb == NTQB - 1) and (hp == NPAIR - 1)
                if hp + 1 < NPAIR and tqb >= NTQB - 2:
                    if tqb == NTQB - 2:
                        next_state, next_chunks = make_qk(hp + 1)
                        state_chunks = next_chunks
                    else:
                        next_chunks = state_chunks
                    piece_sched = ({(NTQB - 2, 3): 0, (NTQB - 2, 7): 1,
                                    (NTQB - 2, 11): 2, (NTQB - 1, 4): 3,
                                    (NTQB - 1, 9): 4})
                else:
                    next_chunks = []
                    piece_sched = {}
                tq_sl = slice(tqb * 512, (tqb + 1) * 512)

                def scores(tsb):
                    ts_sl = slice(tsb * 128, (tsb + 1) * 128)
                    st2 = psum.tile([128, 1024], F32, tag="st", bufs=3, name="st2")
                    for ih in range(2):
                        nc.tensor.matmul(
                            st2[:, ih * 512:(ih + 1) * 512],
                            lhsT=k_pair[ih * 64:(ih + 1) * 64, ts_sl],
                            rhs=q_pair[ih * 64:(ih + 1) * 64, tq_sl],
                            start=True, stop=True,
                        )
                    return st2

                acc_a = psum.tile([128, 512], F32, tag="acc", bufs=2, name="acc_a")
                acc_b = psum.tile([128, 512], F32, tag="acc", bufs=2, name="acc_b")
                sts = [scores(0), scores(1)]
                prs = {}
                for tsb in range(NTSB):
                    # prefetch two blocks ahead so Tensor never waits on
                    # the exp/mask chain
                    if tsb + 2 < NTSB:
                        sts.append(scores(tsb + 2))
                    st_cur = sts.pop(0)
                    if tsb % 2 == 0:
                        prs["pr"] = ppool.tile([128, 2, 1024], BF16,
                                               tag="praw", bufs=3, name="pr2")
                    pr2 = prs["pr"]
                    half = tsb % 2
                    first_blk = (hp == 0 and tqb == 0)
                    dve_exp = ((tsb in (0, 4, 8, 12)) if first_blk
                               else (tsb in (0, 8) and not next_chunks))
                    if dve_exp:
                        # bf16 Schraudolph exp on DVE relieves the Scalar
                        # pacer: bitcast_bf16(int16(x*128/ln2 + 127*128 - 5.5))
                        # ~= e^x within ~4%; int16 saturation maps masked
                        # scores to +/-tiny which the mask multiply zeroes
                        nc.vector.tensor_scalar(
                            out=pr2[:, half, :].bitcast(mybir.dt.int16),
                            in0=st_cur, scalar1=184.66280, scalar2=16250.5,
                            op0=OP.mult, op1=OP.add,
                        )
                    else:
                        nc.scalar.activation(out=pr2[:, half, :], in_=st_cur,
                                             func=AF.Exp)
                    if half == 1:
                        # one DVE op masks two tsb x two heads in place
                        # (mask broadcast via zero-stride head axis)
                        pr4 = pr2.rearrange("p t (h q) -> p t h q", q=512)
                        m4 = mask_s[:, tsb - 1:tsb + 1, tq_sl].rearrange(
                            "p t (o q) -> p t o q", o=1)
                        m4b, _ = bass.broadcast_tensor_aps(m4, pr4)
                        nc.vector.tensor_tensor(pr4, pr4, m4b, OP.mult)
                        for tt in (tsb - 1, tsb):
                            for ih, acc in ((0, acc_a), (1, acc_b)):
                                nc.tensor.matmul(
                                    acc, lhsT=vt[:, tt, 2 * hp + ih, :],
                                    rhs=pr2[:, tt - tsb + 1,
                                            ih * 512:(ih + 1) * 512],
                                    start=(tt == 0), stop=(tt == NTSB - 1),
                                )
                    if norm_q:
                        norm_q.pop(0)()
                    if tsb == NTSB - 1:
                        # free the accumulators with DVE copies into one
                        # double-width tile; normalization drips through the
                        # next block as a single chain
                        accs2 = rpool.tile([128, 1024], BF16, tag="accs",
                                           bufs=2, name="accs2")
                        nc.vector.tensor_copy(out=accs2[:, 0:512], in_=acc_a)
                        nc.vector.tensor_copy(out=accs2[:, 512:1024], in_=acc_b)
                        norm_q.extend(norm_steps((accs2, hp, tq_sl)))
                    # the next head-pair's q/k projections are spread over
                    # all four blocks of this pair in 512-wide pieces
                    if next_chunks and (tqb, tsb) in piece_sched:
                        next_chunks[piece_sched[(tqb, tsb)]]()
                    if final_block:
                        if tsb == 4:
                            wp_dma(0)
                        elif tsb == 8:
                            wp_dma(1)
            if next_state is not None:
                qk_state = next_state

        for step in norm_q:
            step()

        # ---- proj + bias + residual (partial: local 512 a-channels) ----
        # hps spans 2 banks (2 tqb chunks); bank tags rotate st/qk so matmuls
        # never wait on the ot post-processing, which is split DVE/Scalar
        # th outer: the tq[0:1024] half only depends on normalizations that
        # finished before the final block, so its matmuls overlap the norm
        # drain; Scalar (idle here) does all psum reads, DVE adds residuals
        for th in range(2):
            for mb in range(NCO):
                wp_sl = wp_tiles.pop(mb)
                if mb + 2 < NCO:
                    wp_dma(mb + 2)
                elif th == 0:
                    # refill the rotation for the second tq-half pass
                    wp_dma(mb + 2 - NCO)
                hps = psum.tile([128, 1024], F32, tag="st", bufs=3,
                                name="hps")
                for tqb2 in range(2):
                    tq_sl = slice(th * 1024 + tqb2 * 512,
                                  th * 1024 + (tqb2 + 1) * 512)
                    for kb2 in range(NCA // 2):
                        nc.tensor.matmul(
                            hps[:, tqb2 * 512:(tqb2 + 1) * 512],
                            lhsT=wp_sl[:, 2 * kb2:2 * kb2 + 2, :],
                            rhs=a_all[:, 2 * kb2:2 * kb2 + 2, tq_sl],
                            start=(kb2 == 0), stop=(kb2 == NCA // 2 - 1),
                            perf_mode=DR,
                        )
                # hps = (8*Wp) @ (128*a) = 1024*h ; fold 1/1024 + bp/2 here
                t_sl = slice(th * 1024, (th + 1) * 1024)
                ot1 = opool.tile([128, 1024], BF16, tag="ot1")
                nc.scalar.activation(
                    out=ot1, in_=hps, func=AF.Identity,
                    bias=sm[:, S_BP + mb:S_BP + mb + 1],
                    scale=1.0 / 1024.0,
                )
                ot2 = opool.tile([128, 1024], BF16, tag="ot2")
                nc.vector.tensor_tensor(ot2, ot1, xnb[:, mb, t_sl], OP.add)
                deng = nc.sync if mb % 2 == 0 else nc.scalar
                deng.dma_start(out=out_t[:, mb, t_sl], in_=ot2)


_NC_CACHE = None


def _get_nc():
    global _NC_CACHE
    if _NC_CACHE is None:
        _NC_CACHE = build_nc()
    return _NC_CACHE


def _prep_inputs(x, mask, gn_weight, gn_bias, W_qkv, b_qkv, W_proj, b_proj):
    x = np.asarray(x, np.float32)
    mask = np.asarray(mask)
    gnw = np.asarray(gn_weight, np.float32)
    gnb = np.asarray(gn_bias, np.float32)
    W_qkv = np.asarray(W_qkv, np.float32)
    b_qkv = np.asarray(b_qkv, np.float32)
    W_proj = np.asarray(W_proj, np.float32)
    b_proj = np.asarray(b_proj, np.float32)

    Wh = W_qkv.reshape(H, 3, CH, C)
    bh = b_qkv.reshape(H, 3, CH)
    WqT = Wh[:, 0].reshape(C, C).T      # [c, qrow] head-major rows
    WkT = Wh[:, 1].reshape(C, C).T
    WvT = Wh[:, 2].reshape(C, C).T
    WpT = W_proj.T                       # [c_a, out_row]
    bq = bh[:, 0].reshape(C) * SCALE
    bk = bh[:, 1].reshape(C) * SCALE
    bv = bh[:, 2].reshape(C)

    def tile3(w):  # [C, N] -> [128, NCO, N] with c = co*128 + p
        return np.ascontiguousarray(w.reshape(NCO, 128, -1).transpose(1, 0, 2))

    wq3, wk3, wv3, wp3 = tile3(WqT), tile3(WkT), tile3(WvT), tile3(WpT)

    col8 = lambda v: np.ascontiguousarray(v.reshape(8, 128).T)
    col4 = lambda v: np.ascontiguousarray(v.reshape(NPAIR, 128).T)
    ind2 = np.zeros((128, 128), np.float32)
    for j in range(4):
        ind2[32 * j:32 * (j + 1), 32 * j:32 * (j + 1)] = 1.0

    maskT = mask.T.astype(np.float32)   # [ts, tq]
    mask_t = np.ascontiguousarray(
        maskT.reshape(NTSB, 128, T).transpose(1, 0, 2)).astype(ml_dtypes.bfloat16)

    halves = []
    for hh in range(2):
        hsl = slice(hh * 512, (hh + 1) * 512)       # q/k/v row range (8 heads)
        wq_t = (np.ascontiguousarray(
            np.stack([wq3[:, :, hh * 512 + i * 128: hh * 512 + (i + 1) * 128]
                      for i in range(NPAIR)])
        ) * WSCALE).astype(ml_dtypes.float8_e4m3)
        wk_t = (np.ascontiguousarray(
            np.stack([wk3[:, :, hh * 512 + i * 128: hh * 512 + (i + 1) * 128]
                      for i in range(NPAIR)])
        ) * WSCALE).astype(ml_dtypes.float8_e4m3)
        wv_t = (np.ascontiguousarray(wv3[:, :, hsl]) * WSCALE).astype(
            ml_dtypes.float8_e4m3)
        # proj: rows = local 512 a-channels; negative scale folds the
        # Newton-reciprocal sign into the projection
        wp_l = WpT[hsl]                              # [512, C]
        wp3l = np.ascontiguousarray(
            wp_l.reshape(NCA, 128, C).transpose(1, 0, 2))  # [128, NCA, C]
        wp_t = (np.ascontiguousarray(
            np.stack([wp3l[:, :, i * 128:(i + 1) * 128] for i in range(NCO)])
        ) * -WSCALE).astype(ml_dtypes.float8_e4m3)
        halves.append({
            "wq_t": wq_t, "wk_t": wk_t, "wv_t": wv_t, "wp_t": wp_t,
            "bqs_t": col4(bq[hsl]), "bks_t": col4(bk[hsl]),
            "bvb_t": np.ascontiguousarray(
                np.tile(bv[hsl][None, :] * WSCALE, (128, 1))
            ).astype(ml_dtypes.bfloat16),
        })

    common = {
        "gnw_t": col8(gnw), "gnb_t": col8(gnb),
        "bp_t": col8(b_proj * 0.5),
        "ind2_t": ind2,
        "mask_t": mask_t,
    }

    in_maps = []
    for core in range(8):
        b, hh = core // 2, core % 2
        im = dict(common)
        im.update(halves[hh])
        im["x_t"] = np.ascontiguousarray(
            x[b].reshape(NCO, 128, T).transpose(1, 0, 2)).astype(
            ml_dtypes.bfloat16)
        in_maps.append(im)
    return in_maps


def _assemble(results):
    out = np.zeros((B, C, T), np.float32)
    for b in range(B):
        o0 = np.asarray(results[2 * b]["out_t"]).astype(np.float32)
        o1 = np.asarray(results[2 * b + 1]["out_t"]).astype(np.float32)
        o = o0 + o1                                   # [128, NCO, T]
        out[b] = o.transpose(1, 0, 2).reshape(C, T)
    return out


def run(inputs, trace=False, **kw):
    nc = _get_nc()
    in_maps = _prep_inputs(**inputs)
    br = run_bass_kernel_spmd(nc, in_maps, core_ids=list(range(8)), trace=trace, **kw)
    return _assemble(br.results), br


def kernel(**inputs):
    out, _ = run(inputs, trace=False)
    return out


# revision 43
# speedup vs baseline: 1.0043x; 1.0043x over previous
"""Trainium2 Bass kernel for nn_AttentionBlock (B=4, C=1024, T=2048, H=16, GROUPS=32).

Sharding: 8 cores = 4 batches x 2 head-halves (tensor parallel). Each core
computes, for its batch b and its 8 heads:
  - GroupNorm(x[b]) (full T; stats duplicated between the two cores of a pair)
  - q/k/v for its 8 heads over the full T (no duplicated projection work)
  - masked softmax attention for its 8 heads, full T x T
  - partial proj (contraction over its 512 a-channels) + xn/2 + b_proj/2
  - host sums the two partial outputs of each pair (the "all-reduce")

Numerics: GroupNorm stats in fp32; weights/xn/a in fp8e4m3 (DoubleRow matmuls
for qkv/v/proj, x8 weight prescale); q/k/probabilities bf16. Softmax
denominators come free from the PV matmul (v^T tiles carry a 64-wide block of
1/16 so d/16 lands replicated on psum partitions 64:128); -1/d via a bf16
bit-hack seed + one Newton step on DVE (sign folded into W_proj). q/k
projections for the next head pair are emitted inside the previous attention
block's tail; softmax normalization drips one DVE op per tsb through the next
block. Output is bf16 partials summed on the host in fp32.
"""

import numpy as np
import ml_dtypes

import concourse.bass as bass
import concourse.bacc as bacc_mod
import concourse.tile as tile
import concourse.mybir as mybir
from concourse.bass_utils import run_bass_kernel_spmd

F32 = mybir.dt.float32
F32R = mybir.dt.float32r
BF16 = mybir.dt.bfloat16
FP8 = mybir.dt.float8e4
AF = mybir.ActivationFunctionType
OP = mybir.AluOpType
DR = mybir.MatmulPerfMode.DoubleRow
WSCALE = 8.0          # host pre-scales weights into fp8 normal range
ONES_V = 1.0 / 16.0   # ones block value; makes a_all land in fp8 normal range

B, C, T, H = 4, 1024, 2048, 16
GROUPS = 32
EPS = 1e-5
CH = C // H              # 64
SCALE = float(CH) ** -0.25
NH = H // 2              # 8 heads per core
NPAIR = NH // 2          # 4 local head pairs
NCO = C // 128           # 8 channel blocks
NCA = NH * CH // 128     # 4 local a-channel blocks
NTSB = T // 128          # 16 key/value blocks
NTQB = T // 512          # 4 query sub-blocks per pair
INV_N = 1.0 / (32 * T)   # group size = 32 channels x T

# smalls tile column layout (all [128, 8] blocks)
S_S1, S_S2 = 0, 8
S_MU, S_E2, S_VAR, S_SQ, S_RS, S_A, S_B = 16, 24, 32, 40, 48, 56, 64
S_GNW, S_GNB, S_BQ, S_BK, S_BP = 72, 80, 88, 96, 104
S_A2, S_B2 = 112, 120
S_EPS = 128
S_COLS = 129


def build_nc():
    nc = bacc_mod.Bacc(None, target_bir_lowering=False)
    f = {}
    f["x_t"] = nc.dram_tensor("x_t", [128, NCO, T], BF16, kind="ExternalInput")
    f["mask_t"] = nc.dram_tensor("mask_t", [128, NTSB, T], BF16, kind="ExternalInput")
    f["wq_t"] = nc.dram_tensor("wq_t", [NPAIR, 128, NCO, 128], FP8, kind="ExternalInput")
    f["wk_t"] = nc.dram_tensor("wk_t", [NPAIR, 128, NCO, 128], FP8, kind="ExternalInput")
    f["wv_t"] = nc.dram_tensor("wv_t", [128, NCO, 512], FP8, kind="ExternalInput")
    f["wp_t"] = nc.dram_tensor("wp_t", [NCO, 128, NCA, 128], FP8, kind="ExternalInput")
    f["gnw_t"] = nc.dram_tensor("gnw_t", [128, NCO], F32, kind="ExternalInput")
    f["gnb_t"] = nc.dram_tensor("gnb_t", [128, NCO], F32, kind="ExternalInput")
    f["bqs_t"] = nc.dram_tensor("bqs_t", [128, NPAIR], F32, kind="ExternalInput")
    f["bks_t"] = nc.dram_tensor("bks_t", [128, NPAIR], F32, kind="ExternalInput")
    f["bvb_t"] = nc.dram_tensor("bvb_t", [128, 512], BF16, kind="ExternalInput")
    f["bp_t"] = nc.dram_tensor("bp_t", [128, NCO], F32, kind="ExternalInput")
    f["ind2_t"] = nc.dram_tensor("ind2_t", [128, 128], F32R, kind="ExternalInput")
    out_t = nc.dram_tensor("out_t", [128, NCO, T], BF16, kind="ExternalOutput")

    with tile.TileContext(nc) as tc:
        build_body(nc, tc, f, out_t)
    nc.compile()
    return nc


def build_body(nc, tc, f, out_t):
    import contextlib
    ctx = contextlib.ExitStack()
    with ctx:
        singles = ctx.enter_context(tc.tile_pool(name="singles", bufs=1))
        bigp = ctx.enter_context(tc.tile_pool(name="bigp", bufs=1))
        wqk = ctx.enter_context(tc.tile_pool(name="wqk", bufs=3))
        qpool = ctx.enter_context(tc.tile_pool(name="qpool", bufs=2))
        kpool = ctx.enter_context(tc.tile_pool(name="kpool", bufs=2))
        ppool = ctx.enter_context(tc.tile_pool(name="ppool", bufs=3))
        rpool = ctx.enter_context(tc.tile_pool(name="rpool", bufs=2))
        opool = ctx.enter_context(tc.tile_pool(name="opool", bufs=2))
        psum = ctx.enter_context(tc.tile_pool(name="psum", bufs=4, space="PSUM"))

        # ---- persistent tiles ----
        xnb = singles.tile([128, NCO, T], BF16)      # GroupNorm/2 (residual half)
        xn8 = singles.tile([128, NCO, T], FP8)       # GroupNorm output (fp8, matmul input)
        vt = singles.tile([128, NTSB, NH, 128], BF16)
        a_all = singles.tile([128, NPAIR, T], FP8)
        sm = singles.tile([128, S_COLS], F32)
        gst2 = singles.tile([128, 16], F32R)
        bvb = singles.tile([128, 512], BF16)
        ind2 = singles.tile([128, 128], F32R)
        # wv dies once v-proj ends, well before the second q_pair tile is
        # allocated, so it can ride in the qpool rotation
        wv_all = qpool.tile([128, NCO, 512], FP8, tag="qpair", name="wv_all")

        # raw x shares its SBUF slot with the full-T mask, which only starts
        # loading after x is dead
        xs = bigp.tile([128, NCO, T], BF16, tag="big", name="xs")

        # x alternates across the sync + gpsimd DMA rings (Pool is idle this
        # early) so the scalar ring only carries the small coefficient
        # transfers and Scalar can start the stats squares immediately
        nc.scalar.dma_start(out=ind2, in_=f["ind2_t"][:])
        for co in range(NCO):
            eng = nc.sync if co % 2 == 0 else nc.gpsimd
            eng.dma_start(out=xs[:, co, :], in_=f["x_t"][:, co, :])
        # wv rides the scalar ring behind the tiny coefficient transfers so
        # it never delays the x slices that gate the stats
        nc.scalar.dma_start(out=wv_all, in_=f["wv_t"][:])
        nc.scalar.dma_start(out=sm[:, S_GNW:S_GNW + 8], in_=f["gnw_t"][:])
        nc.scalar.dma_start(out=sm[:, S_GNB:S_GNB + 8], in_=f["gnb_t"][:])
        nc.scalar.dma_start(out=bvb, in_=f["bvb_t"][:])
        nc.scalar.dma_start(out=sm[:, S_BQ:S_BQ + NPAIR], in_=f["bqs_t"][:])
        nc.scalar.dma_start(out=sm[:, S_BK:S_BK + NPAIR], in_=f["bks_t"][:])
        nc.scalar.dma_start(out=sm[:, S_BP:S_BP + 8], in_=f["bp_t"][:])

        nc.vector.memset(sm[:, S_EPS:S_EPS + 1], EPS)

        # ---- GroupNorm stats, processed in two co-halves so the first
        # half's xn8 (and thus the v projection) starts ~10us earlier ----
        # per-channel sums on DVE; per-channel sums of squares on Scalar
        # (Square + accumulator) so the two run concurrently
        trash = kpool.tile([128, T], BF16, tag="kpair", name="trash")
        gps_full = psum.tile([128, 512], F32, tag="acc", bufs=2, name="gps")

        def stats_half(hf):
            o = hf * 4
            sl = slice(o, o + 4)
            for co in range(o, o + 4):
                nc.vector.tensor_reduce(
                    out=sm[:, co:co + 1], in_=xs[:, co, :],
                    axis=mybir.AxisListType.X, op=OP.add,
                )
                nc.scalar.activation(
                    out=trash, in_=xs[:, co, :], func=AF.Square,
                    accum_out=sm[:, 8 + co:9 + co],
                )
            # pack this half's 8 stat columns and aggregate the 32-channel
            # groups via the block-diagonal indicator matmul
            g8 = gst2[:, hf * 8:hf * 8 + 8]
            nc.vector.tensor_copy(out=g8[:, 0:4], in_=sm[:, sl])
            nc.vector.tensor_copy(out=g8[:, 4:8], in_=sm[:, 8 + o:8 + o + 4])
            gps = gps_full[:, hf * 16:hf * 16 + 8]
            nc.tensor.matmul(gps, lhsT=ind2, rhs=g8, start=True, stop=True)

            def c(base):
                return sm[:, base + o:base + o + 4]

            nc.vector.tensor_scalar_mul(c(S_MU), gps[:, 0:4], INV_N)
            nc.vector.tensor_scalar_mul(c(S_E2), gps[:, 4:8], INV_N)
            nc.vector.tensor_tensor(c(S_VAR), c(S_MU), c(S_MU), OP.mult)
            nc.vector.tensor_tensor(c(S_VAR), c(S_E2), c(S_VAR), OP.subtract)
            nc.scalar.activation(out=c(S_SQ), in_=c(S_VAR),
                                 func=AF.Sqrt, bias=sm[:, S_EPS:S_EPS + 1])
            nc.vector.reciprocal(out=c(S_RS), in_=c(S_SQ))
            # A = rstd * gn_w ; Bc = gn_b - mu * A ; halved copies for the
            # residual (each core of a pair contributes xn/2)
            nc.vector.tensor_tensor(c(S_A), c(S_RS), c(S_GNW), OP.mult)
            nc.vector.tensor_tensor(c(S_B), c(S_MU), c(S_A), OP.mult)
            nc.vector.tensor_tensor(c(S_B), c(S_GNB), c(S_B), OP.subtract)
            nc.vector.tensor_scalar_mul(c(S_A2), c(S_A), 0.5)
            nc.vector.tensor_scalar_mul(c(S_B2), c(S_B), 0.5)
            for co in range(o, o + 4):
                if co % 2 == 0:
                    nc.vector.tensor_scalar(
                        out=xn8[:, co, :], in0=xs[:, co, :],
                        scalar1=sm[:, S_A + co:S_A + co + 1],
                        scalar2=sm[:, S_B + co:S_B + co + 1],
                        op0=OP.mult, op1=OP.add,
                    )
                else:
                    nc.scalar.activation(
                        out=xn8[:, co, :], in_=xs[:, co, :], func=AF.Identity,
                        bias=sm[:, S_B + co:S_B + co + 1],
                        scale=sm[:, S_A + co:S_A + co + 1],
                    )

        stats_half(0)
        stats_half(1)
        for co in range(NCO):
            nc.gpsimd.tensor_scalar(
                out=xnb[:, co, :], in0=xs[:, co, :],
                scalar1=sm[:, S_A2 + co:S_A2 + co + 1],
                scalar2=sm[:, S_B2 + co:S_B2 + co + 1],
                op0=OP.mult, op1=OP.add,
            )

        # full-T mask loads into the big slot as soon as x dies; the first
        # chunks ride the scalar ring (free once the smalls are done)
        mask_s = bigp.tile([128, NTSB, T], BF16, tag="big", name="mask_s")
        for mc in range(8):
            eng = nc.scalar if mc < 4 else nc.sync
            eng.dma_start(out=mask_s[:, 2 * mc:2 * mc + 2, :],
                          in_=f["mask_t"][:, 2 * mc:2 * mc + 2, :])

        # ---- v^T for the 8 local heads ----
        # vt[:, tsb, h, 0:64] = v^T block; vt[:, tsb, h, 64:128] = ones so the
        # PV matmul also produces the softmax denominator on partitions 64:128
        for mg in range(4):
            nc.gpsimd.memset(vt[:, 4 * mg:4 * mg + 4, :, CH:128], ONES_V)

        vgroups = []

        def v_group(tbg):
            vps = [psum.tile([128, 512], F32, tag=("acc" if i < 2 else "st"),
                             bufs=(2 if i < 2 else 3), name=f"vps{i}")
                   for i in range(4)]
            for kb2 in range(NCO // 2):
                wv_sl = wv_all[:, 2 * kb2:2 * kb2 + 2, :]
                for i in range(4):
                    tb = tbg * 4 + i
                    nc.tensor.matmul(
                        vps[i],
                        lhsT=xn8[:, 2 * kb2:2 * kb2 + 2, tb * 128:(tb + 1) * 128],
                        rhs=wv_sl,
                        start=(kb2 == 0), stop=(kb2 == NCO // 2 - 1),
                        perf_mode=DR,
                    )
            for i in range(4):
                tb = tbg * 4 + i
                nc.vector.tensor_tensor(
                    out=vt[:, tb, :, 0:CH],
                    in0=vps[i].rearrange("p (h c) -> p h c", c=CH),
                    in1=bvb.rearrange("p (h c) -> p h c", c=CH),
                    op=OP.add,
                )

        # ---- per head-pair: q/k projections then attention ----
        norm_q = []

        def norm_steps(item):
            # -1/d via bf16 bit-hack seed + one Newton step, in cheap DVE
            # ALU ops (the iterative InstReciprocal is 4x slower); the sign
            # is folded into W_proj on the host. One double-width chain
            # covers both heads of the block (accs2 cols 0:512 = head even,
            # 512:1024 = head odd), dripped one op per tsb.
            accs_, hp_, tq_sl_ = item
            I16 = mybir.dt.int16
            h = {}

            # scratch tiles are [128, 1024] sliced at [64:128] so SB+SB
            # operands share the same base partition as accs_[64:128]
            def s_t1():
                # NOT(x - 0x7EF3) == (x - 0x7EF2) * -1 in two's complement,
                # so the seed needs only one arith tensor_scalar
                t1f = rpool.tile([128, 1024], I16, tag="rd", bufs=3, name="t1f")
                h["t1"] = t1f[64:128, :]
                nc.vector.tensor_scalar(
                    out=h["t1"], in0=accs_[64:128, :].bitcast(I16),
                    scalar1=0x7EF2, scalar2=-1,
                    op0=OP.subtract, op1=OP.mult,
                )

            def s_u():
                uf = rpool.tile([128, 1024], BF16, tag="rd", bufs=3, name="uf")
                h["u"] = uf[64:128, :]
                nc.vector.tensor_tensor(out=h["u"], in0=accs_[64:128, :],
                                        in1=h["t1"].bitcast(BF16), op=OP.mult)

            def s_rneg():
                # stt computes (scalar op0 in0) op1 in1 = (2 - u) * r0
                rnegf = rpool.tile([128, 1024], BF16, tag="rd", bufs=3,
                                   name="rnegf")
                h["rneg"] = rnegf[0:64, :]
                nc.vector.scalar_tensor_tensor(
                    out=h["rneg"], in0=h["u"], scalar=2.0,
                    in1=h["t1"].bitcast(BF16),
                    op0=OP.subtract, op1=OP.mult,
                )

            def s_fin0():
                nc.gpsimd.tensor_tensor(
                    out=a_all[0:64, hp_, tq_sl_], in0=accs_[0:64, 0:512],
                    in1=h["rneg"][:, 0:512], op=OP.mult,
                )

            def s_fin1():
                nc.gpsimd.tensor_tensor(
                    out=a_all[64:128, hp_, tq_sl_], in0=accs_[0:64, 512:1024],
                    in1=h["rneg"][:, 512:1024], op=OP.mult,
                )

            return [s_t1, s_u, s_rneg, s_fin0, s_fin1]

        def make_qk(hp):
            """q/k projection for head-pair hp as 5 chunks, emitted inside the
            previous attention block (hidden under the exp/mask pipeline) on a
            dedicated PSUM bank pair."""
            state = {}

            def c_dma():
                wq_sl = wqk.tile([128, NCO, 128], FP8, tag="wqkr", name="wq_sl")
                nc.sync.dma_start(out=wq_sl, in_=f["wq_t"][hp])
                wk_sl = wqk.tile([128, NCO, 128], FP8, tag="wqkr", name="wk_sl")
                nc.sync.dma_start(out=wk_sl, in_=f["wk_t"][hp])
                state["wq"], state["wk"] = wq_sl, wk_sl

            def c_qk(th, qu):
                if th == 0 and qu == 0:
                    state["q"] = qpool.tile([128, T], BF16, tag="qpair",
                                            name="q_pair")
                if th == 0 and qu == 1:
                    state["k"] = kpool.tile([128, T], BF16, tag="kpair",
                                            name="k_pair")
                dst = state["q"] if qu == 0 else state["k"]
                w_sl = state["wq"] if qu == 0 else state["wk"]
                bias = sm[:, S_BQ + hp:S_BQ + hp + 1] if qu == 0 else \
                    sm[:, S_BK + hp:S_BK + hp + 1]
                qps2 = psum.tile([128, 1024], F32, tag="st", bufs=3, name="qps2")
                for tqb2 in range(2):
                    for kb2 in range(NCO // 2):
                        nc.tensor.matmul(
                            qps2[:, tqb2 * 512:(tqb2 + 1) * 512],
                            lhsT=w_sl[:, 2 * kb2:2 * kb2 + 2, :],
                            rhs=xn8[:, 2 * kb2:2 * kb2 + 2,
                                    th * 1024 + tqb2 * 512:
                                    th * 1024 + (tqb2 + 1) * 512],
                            start=(kb2 == 0), stop=(kb2 == NCO // 2 - 1),
                            perf_mode=DR,
                        )
                nc.scalar.activation(
                    out=dst[:, th * 1024:(th + 1) * 1024], in_=qps2,
                    func=AF.Identity, bias=bias, scale=SCALE / WSCALE,
                )

            return state, [c_dma, lambda: c_qk(0, 0), lambda: c_qk(1, 0),
                           lambda: c_qk(0, 1), lambda: c_qk(1, 1)]

        # v groups with hp0's q/k pieces interleaved so the q/k DVE copies
        # overlap the later v matmul groups
        qk_state, chunks0 = make_qk(0)
        chunks0[0]()
        v_group(0)
        v_group(1)
        chunks0[1]()
        v_group(2)
        chunks0[2]()
        chunks0[3]()
        v_group(3)
        chunks0[4]()

        wp_tiles = {}

        def wp_dma(mb):
            wp_sl = wqk.tile([128, NCA, 128], FP8, tag="wqkr", name="wp_sl")
            nc.sync.dma_start(out=wp_sl, in_=f["wp_t"][mb])
            wp_tiles[mb] = wp_sl

        for hp in range(NPAIR):
            q_pair, k_pair = qk_state["q"], qk_state["k"]
            next_state = None
            for tqb in range(NTQB):
                final_block = (tqb == NTQB - 1) and (hp == NPAIR - 1)
                if hp + 1 < NPAIR and tqb >= NTQB - 2:
                    if tqb == NTQB - 2:
                        next_state, next_chunks = make_qk(hp + 1)
                        state_chunks = next_chunks
                    else:
                        next_chunks = state_chunks
                    piece_sched = ({(NTQB - 2, 3): 0, (NTQB - 2, 7): 1,
                                    (NTQB - 2, 11): 2, (NTQB - 1, 4): 3,
                                    (NTQB - 1, 9): 4})
                else:
                    next_chunks = []
                    piece_sched = {}
                tq_sl = slice(tqb * 512, (tqb + 1) * 512)

                def scores(tsb):
                    ts_sl = slice(tsb * 128, (tsb + 1) * 128)
                    st2 = psum.tile([128, 1024], F32, tag="st", bufs=3, name="st2")
                    for ih in range(2):
                        nc.tensor.matmul(
                            st2[:, ih * 512:(ih + 1) * 512],
                            lhsT=k_pair[ih * 64:(ih + 1) * 64, ts_sl],
                            rhs=q_pair[ih * 64:(ih + 1) * 64, tq_sl],
                            start=True, stop=True,
                        )
                    return st2

                acc_a = psum.tile([128, 512], F32, tag="acc", bufs=2, name="acc_a")
                acc_b = psum.tile([128, 512], F32, tag="acc", bufs=2, name="acc_b")
                sts = [scores(0), scores(1)]
                prs = {}
                for tsb in range(NTSB):
                    # prefetch two blocks ahead so Tensor never waits on
                    # the exp/mask chain
                    if tsb + 2 < NTSB:
                        sts.append(scores(tsb + 2))
                    st_cur = sts.pop(0)
                    if tsb % 2 == 0:
                        prs["pr"] = ppool.tile([128, 2, 1024], BF16,
                                               tag="praw", bufs=3, name="pr2")
                    pr2 = prs["pr"]
                    half = tsb % 2
                    first_blk = (hp == 0 and tqb == 0)
                    dve_exp = ((tsb in (0, 4, 8, 12)) if first_blk
                               else (tsb in (0, 8)))
                    if dve_exp:
                        # bf16 Schraudolph exp on DVE relieves the Scalar
                        # pacer: bitcast_bf16(int16(x*128/ln2 + 127*128 - 5.5))
                        # ~= e^x within ~4%; int16 saturation maps masked
                        # scores to +/-tiny which the mask multiply zeroes
                        nc.vector.tensor_scalar(
                            out=pr2[:, half, :].bitcast(mybir.dt.int16),
                            in0=st_cur, scalar1=184.66280, scalar2=16250.5,
                            op0=OP.mult, op1=OP.add,
                        )
                    else:
                        nc.scalar.activation(out=pr2[:, half, :], in_=st_cur,
                                             func=AF.Exp)
                    if half == 1:
                        # one DVE op masks two tsb x two heads in place
                        # (mask broadcast via zero-stride head axis)
                        pr4 = pr2.rearrange("p t (h q) -> p t h q", q=512)
                        m4 = mask_s[:, tsb - 1:tsb + 1, tq_sl].rearrange(
                            "p t (o q) -> p t o q", o=1)
                        m4b, _ = bass.broadcast_tensor_aps(m4, pr4)
                        nc.vector.tensor_tensor(pr4, pr4, m4b, OP.mult)
                        for tt in (tsb - 1, tsb):
                            for ih, acc in ((0, acc_a), (1, acc_b)):
                                nc.tensor.matmul(
                                    acc, lhsT=vt[:, tt, 2 * hp + ih, :],
                                    rhs=pr2[:, tt - tsb + 1,
                                            ih * 512:(ih + 1) * 512],
                                    start=(tt == 0), stop=(tt == NTSB - 1),
                                )
                    if norm_q:
                        norm_q.pop(0)()
                    if tsb == NTSB - 1:
                        # free the accumulators with DVE copies into one
                        # double-width tile; normalization drips through the
                        # next block as a single chain
                        accs2 = rpool.tile([128, 1024], BF16, tag="accs",
                                           bufs=2, name="accs2")
                        nc.vector.tensor_copy(out=accs2[:, 0:512], in_=acc_a)
                        nc.vector.tensor_copy(out=accs2[:, 512:1024], in_=acc_b)
                        norm_q.extend(norm_steps((accs2, hp, tq_sl)))
                    # the next head-pair's q/k projections are spread over
                    # all four blocks of this pair in 512-wide pieces
                    if next_chunks and (tqb, tsb) in piece_sched:
                        next_chunks[piece_sched[(tqb, tsb)]]()
                    if final_block:
                        if tsb == 4:
                            wp_dma(0)
                        elif tsb == 8:
                            wp_dma(1)
            if next_state is not None:
                qk_state = next_state

        for step in norm_q:
            step()

        # ---- proj + bias + residual (partial: local 512 a-channels) ----
        # hps spans 2 banks (2 tqb chunks); bank tags rotate st/qk so matmuls
        # never wait on the ot post-processing, which is split DVE/Scalar
        # th outer: the tq[0:1024] half only depends on normalizations that
        # finished before the final block, so its matmuls overlap the norm
        # drain; Scalar (idle here) does all psum reads, DVE adds residuals
        for th in range(2):
            for mb in range(NCO):
                wp_sl = wp_tiles.pop(mb)
                if mb + 2 < NCO:
                    wp_dma(mb + 2)
                elif th == 0:
                    # refill the rotation for the second tq-half pass
                    wp_dma(mb + 2 - NCO)
                hps = psum.tile([128, 1024], F32, tag="st", bufs=3,
                                name="hps")
                for tqb2 in range(2):
                    tq_sl = slice(th * 1024 + tqb2 * 512,
                                  th * 1024 + (tqb2 + 1) * 512)
                    for kb2 in range(NCA // 2):
                        nc.tensor.matmul(
                            hps[:, tqb2 * 512:(tqb2 + 1) * 512],
                            lhsT=wp_sl[:, 2 * kb2:2 * kb2 + 2, :],
                            rhs=a_all[:, 2 * kb2:2 * kb2 + 2, tq_sl],
                            start=(kb2 == 0), stop=(kb2 == NCA // 2 - 1),
                            perf_mode=DR,
                        )
                # hps = (8*Wp) @ (128*a) = 1024*h ; fold 1/1024 + bp/2 here
                t_sl = slice(th * 1024, (th + 1) * 1024)
                ot1 = opool.tile([128, 1024], BF16, tag="ot1")
                nc.scalar.activation(
                    out=ot1, in_=hps, func=AF.Identity,
                    bias=sm[:, S_BP + mb:S_BP + mb + 1],
                    scale=1.0 / 1024.0,
                )
                ot2 = opool.tile([128, 1024], BF16, tag="ot2")
                nc.vector.tensor_tensor(ot2, ot1, xnb[:, mb, t_sl], OP.add)
                deng = nc.sync if mb % 2 == 0 else nc.scalar
                deng.dma_start(out=out_t[:, mb, t_sl], in_=ot2)


_NC_CACHE = None


def _get_nc():
    global _NC_CACHE
    if _NC_CACHE is None:
        _NC_CACHE = build_nc()
    return _NC_CACHE


def _prep_inputs(x, mask, gn_weight, gn_bias, W_qkv, b_qkv, W_proj, b_proj):
    x = np.asarray(x, np.float32)
    mask = np.asarray(mask)
    gnw = np.asarray(gn_weight, np.float32)
    gnb = np.asarray(gn_bias, np.float32)
    W_qkv = np.asarray(W_qkv, np.float32)
    b_qkv = np.asarray(b_qkv, np.float32)
    W_proj = np.asarray(W_proj, np.float32)
    b_proj = np.asarray(b_proj, np.float32)

    Wh = W_qkv.reshape(H, 3, CH, C)
    bh = b_qkv.reshape(H, 3, CH)
    WqT = Wh[:, 0].reshape(C, C).T      # [c, qrow] head-major rows
    WkT = Wh[:, 1].reshape(C, C).T
    WvT = Wh[:, 2].reshape(C, C).T
    WpT = W_proj.T                       # [c_a, out_row]
    bq = bh[:, 0].reshape(C) * SCALE
    bk = bh[:, 1].reshape(C) * SCALE
    bv = bh[:, 2].reshape(C)

    def tile3(w):  # [C, N] -> [128, NCO, N] with c = co*128 + p
        return np.ascontiguousarray(w.reshape(NCO, 128, -1).transpose(1, 0, 2))

    wq3, wk3, wv3, wp3 = tile3(WqT), tile3(WkT), tile3(WvT), tile3(WpT)

    col8 = lambda v: np.ascontiguousarray(v.reshape(8, 128).T)
    col4 = lambda v: np.ascontiguousarray(v.reshape(NPAIR, 128).T)
    ind2 = np.zeros((128, 128), np.float32)
    for j in range(4):
        ind2[32 * j:32 * (j + 1), 32 * j:32 * (j + 1)] = 1.0

    maskT = mask.T.astype(np.float32)   # [ts, tq]
    mask_t = np.ascontiguousarray(
        maskT.reshape(NTSB, 128, T).transpose(1, 0, 2)).astype(ml_dtypes.bfloat16)

    halves = []
    for hh in range(2):
        hsl = slice(hh * 512, (hh + 1) * 512)       # q/k/v row range (8 heads)
        wq_t = (np.ascontiguousarray(
            np.stack([wq3[:, :, hh * 512 + i * 128: hh * 512 + (i + 1) * 128]
                      for i in range(NPAIR)])
        ) * WSCALE).astype(ml_dtypes.float8_e4m3)
        wk_t = (np.ascontiguousarray(
            np.stack([wk3[:, :, hh * 512 + i * 128: hh * 512 + (i + 1) * 128]
                      for i in range(NPAIR)])
        ) * WSCALE).astype(ml_dtypes.float8_e4m3)
        wv_t = (np.ascontiguousarray(wv3[:, :, hsl]) * WSCALE).astype(
            ml_dtypes.float8_e4m3)
        # proj: rows = local 512 a-channels; negative scale folds the
        # Newton-reciprocal sign into the projection
        wp_l = WpT[hsl]                              # [512, C]
        wp3l = np.ascontiguousarray(
            wp_l.reshape(NCA, 128, C).transpose(1, 0, 2))  # [128, NCA, C]
        wp_t = (np.ascontiguousarray(
            np.stack([wp3l[:, :, i * 128:(i + 1) * 128] for i in range(NCO)])
        ) * -WSCALE).astype(ml_dtypes.float8_e4m3)
        halves.append({
            "wq_t": wq_t, "wk_t": wk_t, "wv_t": wv_t, "wp_t": wp_t,
            "bqs_t": col4(bq[hsl]), "bks_t": col4(bk[hsl]),
            "bvb_t": np.ascontiguousarray(
                np.tile(bv[hsl][None, :] * WSCALE, (128, 1))
            ).astype(ml_dtypes.bfloat16),
        })

    common = {
        "gnw_t": col8(gnw), "gnb_t": col8(gnb),
        "bp_t": col8(b_proj * 0.5),
        "ind2_t": ind2,
        "mask_t": mask_t,
    }

    in_maps = []
    for core in range(8):
        b, hh = core // 2, core % 2
        im = dict(common)
        im.update(halves[hh])
        im["x_t"] = np.ascontiguousarray(
            x[b].reshape(NCO, 128, T).transpose(1, 0, 2)).astype(
            ml_dtypes.bfloat16)
        in_maps.append(im)
    return in_maps


def _assemble(results):
    out = np.zeros((B, C, T), np.float32)
    for b in range(B):
        o0 = np.asarray(results[2 * b]["out_t"]).astype(np.float32)
        o1 = np.asarray(results[2 * b + 1]["out_t"]).astype(np.float32)
        o = o0 + o1                                   # [128, NCO, T]
        out[b] = o.transpose(1, 0, 2).reshape(C, T)
    return out


def run(inputs, trace=False, **kw):
    nc = _get_nc()
    in_maps = _prep_inputs(**inputs)
    br = run_bass_kernel_spmd(nc, in_maps, core_ids=list(range(8)), trace=trace, **kw)
    return _assemble(br.results), br


def kernel(**inputs):
    out, _ = run(inputs, trace=False)
    return out


# revision 45
# speedup vs baseline: 1.0043x; 1.0000x over previous
"""Trainium2 Bass kernel for nn_AttentionBlock (B=4, C=1024, T=2048, H=16, GROUPS=32).

Sharding: 8 cores = 4 batches x 2 head-halves (tensor parallel). Each core
computes, for its batch b and its 8 heads:
  - GroupNorm(x[b]) (full T; stats duplicated between the two cores of a pair)
  - q/k/v for its 8 heads over the full T (no duplicated projection work)
  - masked softmax attention for its 8 heads, full T x T
  - partial proj (contraction over its 512 a-channels) + xn/2 + b_proj/2
  - host sums the two partial outputs of each pair (the "all-reduce")

Numerics: GroupNorm stats in fp32; weights/xn/a in fp8e4m3 (DoubleRow matmuls
for qkv/v/proj, x8 weight prescale); q/k/probabilities bf16. Softmax
denominators come free from the PV matmul (v^T tiles carry a 64-wide block of
1/16 so d/16 lands replicated on psum partitions 64:128); -1/d via a bf16
bit-hack seed + one Newton step on DVE (sign folded into W_proj). q/k
projections for the next head pair are emitted inside the previous attention
block's tail; softmax normalization drips one DVE op per tsb through the next
block. Output is bf16 partials summed on the host in fp32.
"""

import numpy as np
import ml_dtypes

import concourse.bass as bass
import concourse.bacc as bacc_mod
import concourse.tile as tile
import concourse.mybir as mybir
from concourse.bass_utils import run_bass_kernel_spmd

F32 = mybir.dt.float32
F32R = mybir.dt.float32r
BF16 = mybir.dt.bfloat16
FP8 = mybir.dt.float8e4
AF = mybir.ActivationFunctionType
OP = mybir.AluOpType
DR = mybir.MatmulPerfMode.DoubleRow
WSCALE = 8.0          # host pre-scales weights into fp8 normal range
ONES_V = 1.0 / 16.0   # ones block value; makes a_all land in fp8 normal range

B, C, T, H = 4, 1024, 2048, 16
GROUPS = 32
EPS = 1e-5
CH = C // H              # 64
SCALE = float(CH) ** -0.25
NH = H // 2              # 8 heads per core
NPAIR = NH // 2          # 4 local head pairs
NCO = C // 128           # 8 channel blocks
NCA = NH * CH // 128     # 4 local a-channel blocks
NTSB = T // 128          # 16 key/value blocks
NTQB = T // 512          # 4 query sub-blocks per pair
INV_N = 1.0 / (32 * T)   # group size = 32 channels x T

# smalls tile column layout (all [128, 8] blocks)
S_S1, S_S2 = 0, 8
S_MU, S_E2, S_VAR, S_SQ, S_RS, S_A, S_B = 16, 24, 32, 40, 48, 56, 64
S_GNW, S_GNB, S_BQ, S_BK, S_BP = 72, 80, 88, 96, 104
S_A2, S_B2 = 112, 120
S_EPS = 128
S_COLS = 129


def build_nc():
    nc = bacc_mod.Bacc(None, target_bir_lowering=False)
    f = {}
    f["x_t"] = nc.dram_tensor("x_t", [128, NCO, T], BF16, kind="ExternalInput")
    f["mask_t"] = nc.dram_tensor("mask_t", [128, NTSB, T], BF16, kind="ExternalInput")
    f["wq_t"] = nc.dram_tensor("wq_t", [NPAIR, 128, NCO, 128], FP8, kind="ExternalInput")
    f["wk_t"] = nc.dram_tensor("wk_t", [NPAIR, 128, NCO, 128], FP8, kind="ExternalInput")
    f["wv_t"] = nc.dram_tensor("wv_t", [128, NCO, 512], FP8, kind="ExternalInput")
    f["wp_t"] = nc.dram_tensor("wp_t", [NCO, 128, NCA, 128], FP8, kind="ExternalInput")
    f["gnw_t"] = nc.dram_tensor("gnw_t", [128, NCO], F32, kind="ExternalInput")
    f["gnb_t"] = nc.dram_tensor("gnb_t", [128, NCO], F32, kind="ExternalInput")
    f["bqs_t"] = nc.dram_tensor("bqs_t", [128, NPAIR], F32, kind="ExternalInput")
    f["bks_t"] = nc.dram_tensor("bks_t", [128, NPAIR], F32, kind="ExternalInput")
    f["bvb_t"] = nc.dram_tensor("bvb_t", [128, 512], BF16, kind="ExternalInput")
    f["bp_t"] = nc.dram_tensor("bp_t", [128, NCO], F32, kind="ExternalInput")
    f["ind2_t"] = nc.dram_tensor("ind2_t", [128, 128], F32R, kind="ExternalInput")
    out_t = nc.dram_tensor("out_t", [128, NCO, T], BF16, kind="ExternalOutput")

    with tile.TileContext(nc) as tc:
        build_body(nc, tc, f, out_t)
    nc.compile()
    return nc


def build_body(nc, tc, f, out_t):
    import contextlib
    ctx = contextlib.ExitStack()
    with ctx:
        singles = ctx.enter_context(tc.tile_pool(name="singles", bufs=1))
        bigp = ctx.enter_context(tc.tile_pool(name="bigp", bufs=1))
        wqk = ctx.enter_context(tc.tile_pool(name="wqk", bufs=3))
        qpool = ctx.enter_context(tc.tile_pool(name="qpool", bufs=2))
        kpool = ctx.enter_context(tc.tile_pool(name="kpool", bufs=2))
        ppool = ctx.enter_context(tc.tile_pool(name="ppool", bufs=3))
        rpool = ctx.enter_context(tc.tile_pool(name="rpool", bufs=2))
        opool = ctx.enter_context(tc.tile_pool(name="opool", bufs=2))
        psum = ctx.enter_context(tc.tile_pool(name="psum", bufs=4, space="PSUM"))

        # ---- persistent tiles ----
        xnb = singles.tile([128, NCO, T], BF16)      # GroupNorm/2 (residual half)
        xn8 = singles.tile([128, NCO, T], FP8)       # GroupNorm output (fp8, matmul input)
        vt = singles.tile([128, NTSB, NH, 128], BF16)
        a_all = singles.tile([128, NPAIR, T], FP8)
        sm = singles.tile([128, S_COLS], F32)
        gst2 = singles.tile([128, 16], F32R)
        bvb = singles.tile([128, 512], BF16)
        ind2 = singles.tile([128, 128], F32R)
        # wv dies once v-proj ends, well before the second q_pair tile is
        # allocated, so it can ride in the qpool rotation
        wv_all = qpool.tile([128, NCO, 512], FP8, tag="qpair", name="wv_all")

        # raw x shares its SBUF slot with the full-T mask, which only starts
        # loading after x is dead
        xs = bigp.tile([128, NCO, T], BF16, tag="big", name="xs")

        # x alternates across the sync + gpsimd DMA rings (Pool is idle this
        # early) so the scalar ring only carries the small coefficient
        # transfers and Scalar can start the stats squares immediately
        nc.scalar.dma_start(out=ind2, in_=f["ind2_t"][:])
        for co in range(NCO):
            eng = nc.sync if co % 2 == 0 else nc.gpsimd
            eng.dma_start(out=xs[:, co, :], in_=f["x_t"][:, co, :])
        # wv rides the scalar ring behind the tiny coefficient transfers so
        # it never delays the x slices that gate the stats
        nc.scalar.dma_start(out=wv_all, in_=f["wv_t"][:])
        nc.scalar.dma_start(out=sm[:, S_GNW:S_GNW + 8], in_=f["gnw_t"][:])
        nc.scalar.dma_start(out=sm[:, S_GNB:S_GNB + 8], in_=f["gnb_t"][:])
        nc.scalar.dma_start(out=bvb, in_=f["bvb_t"][:])
        nc.scalar.dma_start(out=sm[:, S_BQ:S_BQ + NPAIR], in_=f["bqs_t"][:])
        nc.scalar.dma_start(out=sm[:, S_BK:S_BK + NPAIR], in_=f["bks_t"][:])
        nc.scalar.dma_start(out=sm[:, S_BP:S_BP + 8], in_=f["bp_t"][:])

        nc.vector.memset(sm[:, S_EPS:S_EPS + 1], EPS)

        # ---- GroupNorm stats, processed in two co-halves so the first
        # half's xn8 (and thus the v projection) starts ~10us earlier ----
        # per-channel sums on DVE; per-channel sums of squares on Scalar
        # (Square + accumulator) so the two run concurrently
        trash = kpool.tile([128, T], BF16, tag="kpair", name="trash")
        gps_full = psum.tile([128, 512], F32, tag="acc", bufs=2, name="gps")

        def stats_half(hf):
            o = hf * 4
            sl = slice(o, o + 4)
            for co in range(o, o + 4):
                nc.vector.tensor_reduce(
                    out=sm[:, co:co + 1], in_=xs[:, co, :],
                    axis=mybir.AxisListType.X, op=OP.add,
                )
                nc.scalar.activation(
                    out=trash, in_=xs[:, co, :], func=AF.Square,
                    accum_out=sm[:, 8 + co:9 + co],
                )
            # pack this half's 8 stat columns and aggregate the 32-channel
            # groups via the block-diagonal indicator matmul
            g8 = gst2[:, hf * 8:hf * 8 + 8]
            nc.vector.tensor_copy(out=g8[:, 0:4], in_=sm[:, sl])
            nc.vector.tensor_copy(out=g8[:, 4:8], in_=sm[:, 8 + o:8 + o + 4])
            gps = gps_full[:, hf * 16:hf * 16 + 8]
            nc.tensor.matmul(gps, lhsT=ind2, rhs=g8, start=True, stop=True)

            def c(base):
                return sm[:, base + o:base + o + 4]

            nc.vector.tensor_scalar_mul(c(S_MU), gps[:, 0:4], INV_N)
            nc.vector.tensor_scalar_mul(c(S_E2), gps[:, 4:8], INV_N)
            nc.vector.tensor_tensor(c(S_VAR), c(S_MU), c(S_MU), OP.mult)
            nc.vector.tensor_tensor(c(S_VAR), c(S_E2), c(S_VAR), OP.subtract)
            nc.scalar.activation(out=c(S_SQ), in_=c(S_VAR),
                                 func=AF.Sqrt, bias=sm[:, S_EPS:S_EPS + 1])
            nc.vector.reciprocal(out=c(S_RS), in_=c(S_SQ))
            # A = rstd * gn_w ; Bc = gn_b - mu * A ; halved copies for the
            # residual (each core of a pair contributes xn/2)
            nc.vector.tensor_tensor(c(S_A), c(S_RS), c(S_GNW), OP.mult)
            nc.vector.tensor_tensor(c(S_B), c(S_MU), c(S_A), OP.mult)
            nc.vector.tensor_tensor(c(S_B), c(S_GNB), c(S_B), OP.subtract)
            nc.vector.tensor_scalar_mul(c(S_A2), c(S_A), 0.5)
            nc.vector.tensor_scalar_mul(c(S_B2), c(S_B), 0.5)
            for co in range(o, o + 4):
                if co % 2 == 0:
                    nc.vector.tensor_scalar(
                        out=xn8[:, co, :], in0=xs[:, co, :],
                        scalar1=sm[:, S_A + co:S_A + co + 1],
                        scalar2=sm[:, S_B + co:S_B + co + 1],
                        op0=OP.mult, op1=OP.add,
                    )
                else:
                    nc.scalar.activation(
                        out=xn8[:, co, :], in_=xs[:, co, :], func=AF.Identity,
                        bias=sm[:, S_B + co:S_B + co + 1],
                        scale=sm[:, S_A + co:S_A + co + 1],
                    )

        stats_half(0)
        stats_half(1)
        for co in range(NCO):
            nc.gpsimd.tensor_scalar(
                out=xnb[:, co, :], in0=xs[:, co, :],
                scalar1=sm[:, S_A2 + co:S_A2 + co + 1],
                scalar2=sm[:, S_B2 + co:S_B2 + co + 1],
                op0=OP.mult, op1=OP.add,
            )

        # full-T mask loads into the big slot as soon as x dies; the first
        # chunks ride the scalar ring (free once the smalls are done)
        mask_s = bigp.tile([128, NTSB, T], BF16, tag="big", name="mask_s")
        for mc in range(8):
            eng = nc.scalar if mc < 4 else nc.sync
            eng.dma_start(out=mask_s[:, 2 * mc:2 * mc + 2, :],
                          in_=f["mask_t"][:, 2 * mc:2 * mc + 2, :])

        # ---- v^T for the 8 local heads ----
        # vt[:, tsb, h, 0:64] = v^T block; vt[:, tsb, h, 64:128] = ones so the
        # PV matmul also produces the softmax denominator on partitions 64:128
        for mg in range(4):
            nc.gpsimd.memset(vt[:, 4 * mg:4 * mg + 4, :, CH:128], ONES_V)

        vgroups = []

        def v_group(tbg):
            vps = [psum.tile([128, 512], F32, tag=("acc" if i < 2 else "st"),
                             bufs=(2 if i < 2 else 3), name=f"vps{i}")
                   for i in range(4)]
            for kb2 in range(NCO // 2):
                wv_sl = wv_all[:, 2 * kb2:2 * kb2 + 2, :]
                for i in range(4):
                    tb = tbg * 4 + i
                    nc.tensor.matmul(
                        vps[i],
                        lhsT=xn8[:, 2 * kb2:2 * kb2 + 2, tb * 128:(tb + 1) * 128],
                        rhs=wv_sl,
                        start=(kb2 == 0), stop=(kb2 == NCO // 2 - 1),
                        perf_mode=DR,
                    )
            for i in range(4):
                tb = tbg * 4 + i
                nc.vector.tensor_tensor(
                    out=vt[:, tb, :, 0:CH],
                    in0=vps[i].rearrange("p (h c) -> p h c", c=CH),
                    in1=bvb.rearrange("p (h c) -> p h c", c=CH),
                    op=OP.add,
                )

        # ---- per head-pair: q/k projections then attention ----
        norm_q = []

        def norm_steps(item):
            # -1/d via bf16 bit-hack seed + one Newton step, in cheap DVE
            # ALU ops (the iterative InstReciprocal is 4x slower); the sign
            # is folded into W_proj on the host. One double-width chain
            # covers both heads of the block (accs2 cols 0:512 = head even,
            # 512:1024 = head odd), dripped one op per tsb.
            accs_, hp_, tq_sl_ = item
            I16 = mybir.dt.int16
            h = {}

            # scratch tiles are [128, 1024] sliced at [64:128] so SB+SB
            # operands share the same base partition as accs_[64:128]
            def s_t1():
                # NOT(x - 0x7EF3) == (x - 0x7EF2) * -1 in two's complement,
                # so the seed needs only one arith tensor_scalar
                t1f = rpool.tile([128, 1024], I16, tag="rd", bufs=3, name="t1f")
                h["t1"] = t1f[64:128, :]
                nc.vector.tensor_scalar(
                    out=h["t1"], in0=accs_[64:128, :].bitcast(I16),
                    scalar1=0x7EF2, scalar2=-1,
                    op0=OP.subtract, op1=OP.mult,
                )

            def s_u():
                uf = rpool.tile([128, 1024], BF16, tag="rd", bufs=3, name="uf")
                h["u"] = uf[64:128, :]
                nc.vector.tensor_tensor(out=h["u"], in0=accs_[64:128, :],
                                        in1=h["t1"].bitcast(BF16), op=OP.mult)

            def s_rneg():
                # stt computes (scalar op0 in0) op1 in1 = (2 - u) * r0
                rnegf = rpool.tile([128, 1024], BF16, tag="rd", bufs=3,
                                   name="rnegf")
                h["rneg"] = rnegf[0:64, :]
                nc.vector.scalar_tensor_tensor(
                    out=h["rneg"], in0=h["u"], scalar=2.0,
                    in1=h["t1"].bitcast(BF16),
                    op0=OP.subtract, op1=OP.mult,
                )

            def s_fin0():
                nc.gpsimd.tensor_tensor(
                    out=a_all[0:64, hp_, tq_sl_], in0=accs_[0:64, 0:512],
                    in1=h["rneg"][:, 0:512], op=OP.mult,
                )

            def s_fin1():
                nc.gpsimd.tensor_tensor(
                    out=a_all[64:128, hp_, tq_sl_], in0=accs_[0:64, 512:1024],
                    in1=h["rneg"][:, 512:1024], op=OP.mult,
                )

            return [s_t1, s_u, s_rneg, s_fin0, s_fin1]

        def make_qk(hp):
            """q/k projection for head-pair hp as 5 chunks, emitted inside the
            previous attention block (hidden under the exp/mask pipeline) on a
            dedicated PSUM bank pair."""
            state = {}

            def c_dma():
                wq_sl = wqk.tile([128, NCO, 128], FP8, tag="wqkr", name="wq_sl")
                nc.sync.dma_start(out=wq_sl, in_=f["wq_t"][hp])
                wk_sl = wqk.tile([128, NCO, 128], FP8, tag="wqkr", name="wk_sl")
                nc.sync.dma_start(out=wk_sl, in_=f["wk_t"][hp])
                state["wq"], state["wk"] = wq_sl, wk_sl

            def c_qk(th, qu):
                if th == 0 and qu == 0:
                    state["q"] = qpool.tile([128, T], BF16, tag="qpair",
                                            name="q_pair")
                if th == 0 and qu == 1:
                    state["k"] = kpool.tile([128, T], BF16, tag="kpair",
                                            name="k_pair")
                dst = state["q"] if qu == 0 else state["k"]
                w_sl = state["wq"] if qu == 0 else state["wk"]
                bias = sm[:, S_BQ + hp:S_BQ + hp + 1] if qu == 0 else \
                    sm[:, S_BK + hp:S_BK + hp + 1]
                qps2 = psum.tile([128, 1024], F32, tag="st", bufs=3, name="qps2")
                for tqb2 in range(2):
                    for kb2 in range(NCO // 2):
                        nc.tensor.matmul(
                            qps2[:, tqb2 * 512:(tqb2 + 1) * 512],
                            lhsT=w_sl[:, 2 * kb2:2 * kb2 + 2, :],
                            rhs=xn8[:, 2 * kb2:2 * kb2 + 2,
                                    th * 1024 + tqb2 * 512:
                                    th * 1024 + (tqb2 + 1) * 512],
                            start=(kb2 == 0), stop=(kb2 == NCO // 2 - 1),
                            perf_mode=DR,
                        )
                nc.scalar.activation(
                    out=dst[:, th * 1024:(th + 1) * 1024], in_=qps2,
                    func=AF.Identity, bias=bias, scale=SCALE / WSCALE,
                )

            return state, [c_dma, lambda: c_qk(0, 0), lambda: c_qk(1, 0),
                           lambda: c_qk(0, 1), lambda: c_qk(1, 1)]

        # v groups with hp0's q/k pieces interleaved so the q/k DVE copies
        # overlap the later v matmul groups
        qk_state, chunks0 = make_qk(0)
        chunks0[0]()
        v_group(0)
        v_group(1)
        chunks0[1]()
        v_group(2)
        chunks0[2]()
        chunks0[3]()
        v_group(3)
        chunks0[4]()

        wp_tiles = {}
        proj_done = set()

        def wp_dma(mb):
            wp_sl = wqk.tile([128, NCA, 128], FP8, tag="wqkr", name="wp_sl")
            nc.sync.dma_start(out=wp_sl, in_=f["wp_t"][mb])
            wp_tiles[mb] = wp_sl

        def proj_group_body(wp_sl, mb, th):
            hps = psum.tile([128, 1024], F32, tag="st", bufs=3, name="hps")
            for tqb2 in range(2):
                tq_sl2 = slice(th * 1024 + tqb2 * 512,
                               th * 1024 + (tqb2 + 1) * 512)
                for kb2 in range(NCA // 2):
                    nc.tensor.matmul(
                        hps[:, tqb2 * 512:(tqb2 + 1) * 512],
                        lhsT=wp_sl[:, 2 * kb2:2 * kb2 + 2, :],
                        rhs=a_all[:, 2 * kb2:2 * kb2 + 2, tq_sl2],
                        start=(kb2 == 0), stop=(kb2 == NCA // 2 - 1),
                        perf_mode=DR,
                    )
            # hps = (8*Wp) @ (128*a) = 1024*h ; fold 1/1024 + bp/2 here
            t_sl = slice(th * 1024, (th + 1) * 1024)
            ot1 = opool.tile([128, 1024], BF16, tag="ot1")
            nc.scalar.activation(
                out=ot1, in_=hps, func=AF.Identity,
                bias=sm[:, S_BP + mb:S_BP + mb + 1],
                scale=1.0 / 1024.0,
            )
            ot2 = opool.tile([128, 1024], BF16, tag="ot2")
            nc.vector.tensor_tensor(ot2, ot1, xnb[:, mb, t_sl], OP.add)
            deng = nc.sync if mb % 2 == 0 else nc.scalar
            deng.dma_start(out=out_t[:, mb, t_sl], in_=ot2)

        def proj_group(mb, th):
            proj_done.add((mb, th))
            proj_group_body(wp_tiles.pop(mb), mb, th)

        for hp in range(NPAIR):
            q_pair, k_pair = qk_state["q"], qk_state["k"]
            next_state = None
            for tqb in range(NTQB):
                final_block = (tqb == NTQB - 1) and (hp == NPAIR - 1)
                if hp + 1 < NPAIR and tqb >= NTQB - 2:
                    if tqb == NTQB - 2:
                        next_state, next_chunks = make_qk(hp + 1)
                        state_chunks = next_chunks
                    else:
                        next_chunks = state_chunks
                    piece_sched = ({(NTQB - 2, 3): 0, (NTQB - 2, 7): 1,
                                    (NTQB - 2, 11): 2, (NTQB - 1, 4): 3,
                                    (NTQB - 1, 9): 4})
                else:
                    next_chunks = []
                    piece_sched = {}
                tq_sl = slice(tqb * 512, (tqb + 1) * 512)

                def scores(tsb):
                    ts_sl = slice(tsb * 128, (tsb + 1) * 128)
                    st2 = psum.tile([128, 1024], F32, tag="st", bufs=3, name="st2")
                    for ih in range(2):
                        nc.tensor.matmul(
                            st2[:, ih * 512:(ih + 1) * 512],
                            lhsT=k_pair[ih * 64:(ih + 1) * 64, ts_sl],
                            rhs=q_pair[ih * 64:(ih + 1) * 64, tq_sl],
                            start=True, stop=True,
                        )
                    return st2

                acc_a = psum.tile([128, 512], F32, tag="acc", bufs=2, name="acc_a")
                acc_b = psum.tile([128, 512], F32, tag="acc", bufs=2, name="acc_b")
                sts = [scores(0), scores(1)]
                prs = {}
                for tsb in range(NTSB):
                    # prefetch two blocks ahead so Tensor never waits on
                    # the exp/mask chain
                    if tsb + 2 < NTSB:
                        sts.append(scores(tsb + 2))
                    st_cur = sts.pop(0)
                    if tsb % 2 == 0:
                        prs["pr"] = ppool.tile([128, 2, 1024], BF16,
                                               tag="praw", bufs=3, name="pr2")
                    pr2 = prs["pr"]
                    half = tsb % 2
                    first_blk = (hp == 0 and tqb == 0)
                    dve_exp = ((tsb in (0, 4, 8, 12)) if first_blk
                               else (tsb in (0, 8)))
                    if dve_exp:
                        # bf16 Schraudolph exp on DVE relieves the Scalar
                        # pacer: bitcast_bf16(int16(x*128/ln2 + 127*128 - 5.5))
                        # ~= e^x within ~4%; int16 saturation maps masked
                        # scores to +/-tiny which the mask multiply zeroes
                        nc.vector.tensor_scalar(
                            out=pr2[:, half, :].bitcast(mybir.dt.int16),
                            in0=st_cur, scalar1=184.66280, scalar2=16250.5,
                            op0=OP.mult, op1=OP.add,
                        )
                    else:
                        nc.scalar.activation(out=pr2[:, half, :], in_=st_cur,
                                             func=AF.Exp)
                    if half == 1:
                        # one DVE op masks two tsb x two heads in place
                        # (mask broadcast via zero-stride head axis)
                        pr4 = pr2.rearrange("p t (h q) -> p t h q", q=512)
                        m4 = mask_s[:, tsb - 1:tsb + 1, tq_sl].rearrange(
                            "p t (o q) -> p t o q", o=1)
                        m4b, _ = bass.broadcast_tensor_aps(m4, pr4)
                        nc.vector.tensor_tensor(pr4, pr4, m4b, OP.mult)
                        for tt in (tsb - 1, tsb):
                            for ih, acc in ((0, acc_a), (1, acc_b)):
                                nc.tensor.matmul(
                                    acc, lhsT=vt[:, tt, 2 * hp + ih, :],
                                    rhs=pr2[:, tt - tsb + 1,
                                            ih * 512:(ih + 1) * 512],
                                    start=(tt == 0), stop=(tt == NTSB - 1),
                                )
                    if norm_q:
                        norm_q.pop(0)()
                    if tsb == NTSB - 1:
                        # free the accumulators with DVE copies into one
                        # double-width tile; normalization drips through the
                        # next block as a single chain
                        accs2 = rpool.tile([128, 1024], BF16, tag="accs",
                                           bufs=2, name="accs2")
                        nc.vector.tensor_copy(out=accs2[:, 0:512], in_=acc_a)
                        nc.vector.tensor_copy(out=accs2[:, 512:1024], in_=acc_b)
                        norm_q.extend(norm_steps((accs2, hp, tq_sl)))
                    # the next head-pair's q/k projections are spread over
                    # all four blocks of this pair in 512-wide pieces
                    if next_chunks and (tqb, tsb) in piece_sched:
                        next_chunks[piece_sched[(tqb, tsb)]]()
                    if final_block:
                        if tsb == 4:
                            wp_dma(0)
                        elif tsb == 8:
                            wp_dma(1)
                        elif tsb in (10, 13):
                            # pull a proj tq-half-0 group into this block's
                            # Tensor slack; its a_all inputs are already
                            # normalized (tq 0:1024 finished last block)
                            proj_group(tsb == 13, 0)
            if next_state is not None:
                qk_state = next_state

        for step in norm_q:
            step()

        # ---- proj + bias + residual (partial: local 512 a-channels) ----
        # hps spans 2 banks (2 tqb chunks); bank tags rotate st/qk so matmuls
        # never wait on the ot post-processing, which is split DVE/Scalar
        # th outer: the tq[0:1024] half only depends on normalizations that
        # finished before the final block, so its matmuls overlap the norm
        # drain; Scalar (idle here) does all psum reads, DVE adds residuals
        wp_dma(2)
        wp_dma(3)
        for th in range(2):
            for mb in range(NCO):
                if (mb, th) in proj_done:
                    continue
                wp_sl = wp_tiles.pop(mb)
                if mb + 2 < NCO:
                    wp_dma(mb + 2)
                elif th == 0:
                    # refill the rotation for the second tq-half pass
                    wp_dma(mb + 2 - NCO)
                proj_group_body(wp_sl, mb, th)


_NC_CACHE = None


def _get_nc():
    global _NC_CACHE
    if _NC_CACHE is None:
        _NC_CACHE = build_nc()
    return _NC_CACHE


def _prep_inputs(x, mask, gn_weight, gn_bias, W_qkv, b_qkv, W_proj, b_proj):
    x = np.asarray(x, np.float32)
    mask = np.asarray(mask)
    gnw = np.asarray(gn_weight, np.float32)
    gnb = np.asarray(gn_bias, np.float32)
    W_qkv = np.asarray(W_qkv, np.float32)
    b_qkv = np.asarray(b_qkv, np.float32)
    W_proj = np.asarray(W_proj, np.float32)
    b_proj = np.asarray(b_proj, np.float32)

    Wh = W_qkv.reshape(H, 3, CH, C)
    bh = b_qkv.reshape(H, 3, CH)
    WqT = Wh[:, 0].reshape(C, C).T      # [c, qrow] head-major rows
    WkT = Wh[:, 1].reshape(C, C).T
    WvT = Wh[:, 2].reshape(C, C).T
    WpT = W_proj.T                       # [c_a, out_row]
    bq = bh[:, 0].reshape(C) * SCALE
    bk = bh[:, 1].reshape(C) * SCALE
    bv = bh[:, 2].reshape(C)

    def tile3(w):  # [C, N] -> [128, NCO, N] with c = co*128 + p
        return np.ascontiguousarray(w.reshape(NCO, 128, -1).transpose(1, 0, 2))

    wq3, wk3, wv3, wp3 = tile3(WqT), tile3(WkT), tile3(WvT), tile3(WpT)

    col8 = lambda v: np.ascontiguousarray(v.reshape(8, 128).T)
    col4 = lambda v: np.ascontiguousarray(v.reshape(NPAIR, 128).T)
    ind2 = np.zeros((128, 128), np.float32)
    for j in range(4):
        ind2[32 * j:32 * (j + 1), 32 * j:32 * (j + 1)] = 1.0

    maskT = mask.T.astype(np.float32)   # [ts, tq]
    mask_t = np.ascontiguousarray(
        maskT.reshape(NTSB, 128, T).transpose(1, 0, 2)).astype(ml_dtypes.bfloat16)

    halves = []
    for hh in range(2):
        hsl = slice(hh * 512, (hh + 1) * 512)       # q/k/v row range (8 heads)
        wq_t = (np.ascontiguousarray(
            np.stack([wq3[:, :, hh * 512 + i * 128: hh * 512 + (i + 1) * 128]
                      for i in range(NPAIR)])
        ) * WSCALE).astype(ml_dtypes.float8_e4m3)
        wk_t = (np.ascontiguousarray(
            np.stack([wk3[:, :, hh * 512 + i * 128: hh * 512 + (i + 1) * 128]
                      for i in range(NPAIR)])
        ) * WSCALE).astype(ml_dtypes.float8_e4m3)
        wv_t = (np.ascontiguousarray(wv3[:, :, hsl]) * WSCALE).astype(
            ml_dtypes.float8_e4m3)
        # proj: rows = local 512 a-channels; negative scale folds the
        # Newton-reciprocal sign into the projection
        wp_l = WpT[hsl]                              # [512, C]
        wp3l = np.ascontiguousarray(
            wp_l.reshape(NCA, 128, C).transpose(1, 0, 2))  # [128, NCA, C]
        wp_t = (np.ascontiguousarray(
            np.stack([wp3l[:, :, i * 128:(i + 1) * 128] for i in range(NCO)])
        ) * -WSCALE).astype(ml_dtypes.float8_e4m3)
        halves.append({
            "wq_t": wq_t, "wk_t": wk_t, "wv_t": wv_t, "wp_t": wp_t,
            "bqs_t": col4(bq[hsl]), "bks_t": col4(bk[hsl]),
            "bvb_t": np.ascontiguousarray(
                np.tile(bv[hsl][None, :] * WSCALE, (128, 1))
            ).astype(ml_dtypes.bfloat16),
        })

    common = {
        "gnw_t": col8(gnw), "gnb_t": col8(gnb),
        "bp_t": col8(b_proj * 0.5),
        "ind2_t": ind2,
        "mask_t": mask_t,
    }

    in_maps = []
    for core in range(8):
        b, hh = core // 2, core % 2
        im = dict(common)
        im.update(halves[hh])
        im["x_t"] = np.ascontiguousarray(
            x[b].reshape(NCO, 128, T).transpose(1, 0, 2)).astype(
            ml_dtypes.bfloat16)
        in_maps.append(im)
    return in_maps


def _assemble(results):
    out = np.zeros((B, C, T), np.float32)
    for b in range(B):
        o0 = np.asarray(results[2 * b]["out_t"]).astype(np.float32)
        o1 = np.asarray(results[2 * b + 1]["out_t"]).astype(np.float32)
        o = o0 + o1                                   # [128, NCO, T]
        out[b] = o.transpose(1, 0, 2).reshape(C, T)
    return out


def run(inputs, trace=False, **kw):
    nc = _get_nc()
    in_maps = _prep_inputs(**inputs)
    br = run_bass_kernel_spmd(nc, in_maps, core_ids=list(range(8)), trace=trace, **kw)
    return _assemble(br.results), br


def kernel(**inputs):
    out, _ = run(inputs, trace=False)
    return out
